# revision 1
# baseline (speedup 1.0000x reference)
"""2D DCT-II (separable) kernel for Trainium2, data-parallel over 8 NeuronCores.

Problem: img [128, 1, 512, 512] f32 -> out [128, 1, 512, 512] f32 with
    out[b,0] = scale * (Cp @ img[b,0] @ Cq^T)
where Cp[p,m] = cos(pi*(2m+1)*p/1024), Cq[q,n] = cos(pi*(2n+1)*q/1024) and
scale[p,q] = (2/512)*row[p]*col[q] (1/sqrt2 on p==0 / q==0). Since M=N=512 the
two basis matrices are identical; the rank-1 scale is folded into them:
    C'[k,j] = s_k * cos(pi*(2j+1)*k/1024),  s_k = sqrt(2/512) * (1/sqrt2 if k==0 else 1)
    out[b] = C' @ img[b] @ C'^T

Per-core (16 images each): two PE matmul stages with the image/intermediate as
the stationary operand (both stages contract over the data's partition dim, so
no transposes are needed):
    stage1: Dt[n, p] = sum_m A[m, n] * C'T[m, p]   (lhsT = A tile, rhs = C'T)
    stage2: Y[p, q]  = sum_n Dt[n, p] * C'T[n, q]  (lhsT = Dt tile, rhs = C'T)
Matmuls run in float32r (TF32-like, ~11 mantissa bits) at full PE rate.

Stage 1 exploits the DCT-II reflection symmetry C'[p, 511-m] = (-1)^p C'[p, m]:
with E[m'] = A[m'] + A[511-m'] and O[m'] = A[m'] - A[511-m'] (m' < 256),
even output rows come from a 256-contraction with E, odd rows from O —
half the stage-1 PE cycles. The host passes the image as two row-halves
(bottom half row-reversed) so the fold pairs are partition-aligned; E/O are
formed on DVE/GpSimd; the even/odd output-row interleave happens inside the
PSUM->SBUF copies (stride-2 writes, same 1x copy cost).
"""

import sys
import numpy as np

for _p in ("/opt/trn_rl_repo", "/opt/pypackages"):
    if _p not in sys.path:
        sys.path.append(_p)

import concourse.tile as tile  # noqa: E402
from concourse import bacc, mybir  # noqa: E402
from concourse.bass_utils import run_bass_kernel_spmd  # noqa: E402

N_CORES = 8
B_FULL = 128
S = 512  # image side
H = S // 2
BPC = B_FULL // N_CORES  # images per core
T = S // 128  # 4 partition tiles per image side


def _basis_f32() -> np.ndarray:
    """C'T[j, k] = s_k * cos(pi*(2j+1)*k/1024), shape [512, 512] f32."""
    j = np.arange(S, dtype=np.float64)
    k = np.arange(S, dtype=np.float64)
    c = np.cos(np.pi * (2.0 * j[:, None] + 1.0) * k[None, :] / (2.0 * S))
    s = np.full(S, np.sqrt(2.0 / S), dtype=np.float64)
    s[0] /= np.sqrt(2.0)
    return (c * s[None, :]).astype(np.float32)


def _build():
    nc = bacc.Bacc("TRN2", target_bir_lowering=False, debug=False)
    # Image passed as two halves: rows 0:256 forward, rows 511:255 reversed
    # (host-side flip) so fold pairs m' <-> 511-m' are partition-aligned with
    # plain positive-stride DMAs.
    imga_d = nc.dram_tensor(
        "imga", [BPC, H, S], mybir.dt.float32r, kind="ExternalInput"
    ).ap()
    imgb_d = nc.dram_tensor(
        "imgb", [BPC, H, S], mybir.dt.float32r, kind="ExternalInput"
    ).ap()
    ct_d = nc.dram_tensor("ct", [S, S], mybir.dt.float32r, kind="ExternalInput").ap()
    ce_d = nc.dram_tensor("ce", [H, H], mybir.dt.float32r, kind="ExternalInput").ap()
    co_d = nc.dram_tensor("co", [H, H], mybir.dt.float32r, kind="ExternalInput").ap()
    out_d = nc.dram_tensor("out", [BPC, S, S], mybir.dt.float32, kind="ExternalOutput").ap()

    out_v = out_d.rearrange("b (t p) q -> b p t q", p=128)
    imga_v = imga_d.rearrange("b (t p) n -> b p t n", p=128)
    imgb_v = imgb_d.rearrange("b (t p) n -> b p t n", p=128)
    ct_v = ct_d.rearrange("(t p) k -> t p k", p=128)
    ce_v = ce_d.rearrange("(t p) k -> t p k", p=128)
    co_v = co_d.rearrange("(t p) k -> t p k", p=128)

    with tile.TileContext(nc) as tc:
        with (
            tc.tile_pool(name="const", bufs=1) as cpool,
            tc.tile_pool(name="a", bufs=10) as apool,
            tc.tile_pool(name="a01", bufs=8) as a01pool,
            tc.tile_pool(name="eo", bufs=16) as eopool,
            tc.tile_pool(name="dt", bufs=2) as dtpool,
            tc.tile_pool(name="o", bufs=8) as opool,
            tc.tile_pool(name="ps1", bufs=4, space="PSUM") as ps1pool,
            tc.tile_pool(name="ps2", bufs=4, space="PSUM") as ps2pool,
        ):
            # ce tile 0 first — the very first matmul needs only it plus
            # image 0's two halves; the remaining constants follow them.
            ce_sb = cpool.tile([128, 2, H], mybir.dt.float32r)
            co_sb = cpool.tile([128, 2, H], mybir.dt.float32r)
            ct_sb = cpool.tile([128, T, S], mybir.dt.float32r)
            nc.sync.dma_start(co_sb[:, 0, :], co_v[0])

            def emit_load_and_folds(i):
                """DMA image i's halves and emit the E/O folds; returns (e_t, o_t)."""
                if i < 2:
                    # Per-half loads in separate tiles: the first fold (and so
                    # the first matmul) starts after 512 KB instead of 1 MB.
                    ah = []
                    for mh, src in (
                        (0, imga_v[i, :, 0, :]),
                        (1, imgb_v[i, :, 0, :]),
                        (2, imga_v[i, :, 1, :]),
                        (3, imgb_v[i, :, 1, :]),
                    ):
                        t = a01pool.tile(
                            [128, S], mybir.dt.float32r, tag="a01", name=f"ah_{i}_{mh}"
                        )
                        nc.sync.dma_start(t[:], src)
                        ah.append(t)
                    af0, ar0, af1, ar1 = ah[0][:, :], ah[1][:, :], ah[2][:, :], ah[3][:, :]
                else:
                    af = apool.tile([128, 2, S], mybir.dt.float32r, tag="a", name=f"af_{i}")
                    ar = apool.tile([128, 2, S], mybir.dt.float32r, tag="a", name=f"ar_{i}")
                    nc.sync.dma_start(af[:], imga_v[i])
                    nc.sync.dma_start(ar[:], imgb_v[i])
                    af0, ar0, af1, ar1 = af[:, 0, :], ar[:, 0, :], af[:, 1, :], ar[:, 1, :]
                if i == 0:
                    # Remaining constants, ordered by first use.
                    nc.sync.dma_start(co_sb[:, 1, :], co_v[1])
                    for t in range(2):
                        nc.sync.dma_start(ce_sb[:, t, :], ce_v[t])
                    for t in range(T):
                        nc.sync.dma_start(ct_sb[:, t, :], ct_v[t])

                e0 = eopool.tile([128, S], mybir.dt.float32r, tag="eo", name=f"e0_{i}")
                e1 = eopool.tile([128, S], mybir.dt.float32r, tag="eo", name=f"e1_{i}")
                o0 = eopool.tile([128, S], mybir.dt.float32r, tag="eo", name=f"o0_{i}")
                o1 = eopool.tile([128, S], mybir.dt.float32r, tag="eo", name=f"o1_{i}")
                nc.gpsimd.tensor_sub(o0[:], af0, ar0)
                nc.gpsimd.tensor_sub(o1[:], af1, ar1)
                nc.vector.tensor_add(e0[:], af0, ar0)
                nc.vector.tensor_add(e1[:], af1, ar1)
                return (e0, e1), (o0, o1)

            folds = emit_load_and_folds(0)
            for i in range(BPC):
                e_t, o_t = folds

                # stage 1 (folded): Dt[n, 2k] from E/ce, Dt[n, 2k+1] from O/co.
                # ps1[nt] cols 0:256 hold even-p, cols 256:512 odd-p.
                ps1 = [ps1pool.tile([128, S], mybir.dt.float32, tag="ps1", name=f"ps1_{i}_{j}") for j in range(T)]
                # O-part first: the gpsimd subs are ready earlier than the DVE
                # adds (which queue behind the previous image's mid copies).
                for nt in range(T):
                    nts = slice(nt * 128, (nt + 1) * 128)
                    for mh in range(2):
                        nc.tensor.matmul(
                            ps1[nt][:, H:S],
                            o_t[mh][:, nts],
                            co_sb[:, mh, :],
                            start=(mh == 0),
                            stop=(mh == 1),
                        )
                    for mh in range(2):
                        nc.tensor.matmul(
                            ps1[nt][:, 0:H],
                            e_t[mh][:, nts],
                            ce_sb[:, mh, :],
                            start=(mh == 0),
                            stop=(mh == 1),
                        )
                # Prefetch the NEXT image's loads + folds now, so its DVE adds
                # run ahead of this image's mid copies in the DVE queue (the
                # folds were the once-per-image PE stall in the trace).
                if i + 1 < BPC:
                    folds = emit_load_and_folds(i + 1)

                dt_sb = dtpool.tile([128, T, S], mybir.dt.float32r, tag="dt")
                for nt in range(T):
                    # One mid-copy pair on ACT to keep DVE under the PE span.
                    eng = nc.scalar.copy if nt == 3 else nc.vector.tensor_copy
                    eng(dt_sb[:, nt, 0:S:2], ps1[nt][:, 0:H])
                    eng(dt_sb[:, nt, 1:S:2], ps1[nt][:, H:S])

                # stage 2 (p-outer): Y[p, q] = sum_n Dt[n, p] C'T[n, q]
                # Output staged in 2-tile chunks: fewer DMA descriptors while
                # keeping the drain pipelined.
                last = i == BPC - 1
                for ph in range(2):
                    o_sb = opool.tile(
                        [128, 2, S], mybir.dt.float32, tag="o", name=f"o_{i}_{ph}"
                    )
                    for pj in range(2):
                        pt = ph * 2 + pj
                        ps2 = ps2pool.tile(
                            [128, S], mybir.dt.float32, tag="ps2", name=f"ps2_{i}_{pt}"
                        )
                        for nt in range(T):
                            nc.tensor.matmul(
                                ps2[:],
                                dt_sb[:, nt, pt * 128 : (pt + 1) * 128],
                                ct_sb[:, nt, :],
                                start=(nt == 0),
                                stop=(nt == T - 1),
                            )
                        nc.scalar.copy(o_sb[:, pj, :], ps2[:])
                        if last:
                            # Drain the final image per p-tile on alternating
                            # queues so the tail DMA overlaps the last matmuls.
                            eng = nc.scalar if pt % 2 == 0 else nc.sync
                            eng.dma_start(out_v[i, :, pt, :], o_sb[:, pj, :])
                    if not last:
                        if ph == 0:
                            nc.scalar.dma_start(out_v[i, :, 0:2, :], o_sb[:])
                        else:
                            nc.sync.dma_start(out_v[i, :, 2:4, :], o_sb[:])
    nc.compile()
    return nc


_NC_CACHE = None


def _get_nc():
    global _NC_CACHE
    if _NC_CACHE is None:
        _NC_CACHE = _build()
    return _NC_CACHE


def run_sharded(img: np.ndarray, **spmd_kwargs):
    """img [128, 1, 512, 512] f32 -> (out [128, 1, 512, 512] f32, BassKernelResults)."""
    img = np.ascontiguousarray(np.asarray(img, dtype=np.float32)).reshape(B_FULL, S, S)
    imga = np.ascontiguousarray(img[:, :H, :])
    imgb = np.ascontiguousarray(img[:, :H - 1 :-1, :])  # rows 511..256 reversed
    ct = _basis_f32()
    ce = np.ascontiguousarray(ct[:H, 0::2])
    co = np.ascontiguousarray(ct[:H, 1::2])
    nc = _get_nc()
    in_maps = [
        {
            "imga": imga[k * BPC : (k + 1) * BPC],
            "imgb": imgb[k * BPC : (k + 1) * BPC],
            "ct": ct,
            "ce": ce,
            "co": co,
        }
        for k in range(N_CORES)
    ]
    res = run_bass_kernel_spmd(nc, in_maps, core_ids=list(range(N_CORES)), **spmd_kwargs)
    out = np.empty((B_FULL, S, S), dtype=np.float32)
    for k in range(N_CORES):
        out[k * BPC : (k + 1) * BPC] = res.results[k]["out"]
    return out.reshape(B_FULL, 1, S, S), res


def kernel(img: np.ndarray) -> np.ndarray:
    out, _ = run_sharded(img)
    return out



# revision 6
# speedup vs baseline: 1.3412x; 1.3412x over previous
"""2D DCT-II (separable) kernel for Trainium2, data-parallel over 8 NeuronCores.

Problem: img [128, 1, 512, 512] f32 -> out [128, 1, 512, 512] f32 with
    out[b,0] = scale * (Cp @ img[b,0] @ Cq^T),  Cp == Cq, scale rank-1 folded in:
    C'[k,j] = s_k * cos(pi*(2j+1)*k/1024),  s_k = sqrt(2/512) * (1/sqrt2 if k==0)
    out[b] = C' @ A @ C'^T

Scheme (all bf16, rel-err budget 2e-2 >> bf16's ~2e-3):
The DCT reflection symmetry C'[k, 511-j] = (-1)^k C'[k, j] folds BOTH tensor
contractions from 512 to 256 ("quadrant folding"), halving PE work vs the
unfolded separable transform:
    rowE/rowO = A[j] +/- A[511-j]           (rows, level-1)
    A_ee/A_eo/A_oe/A_oo = rowX[:, n'] +/- rowX[:, 511-n']   (cols)
    stage1: Dt_rc[n', p'] = sum_{m'<256} A_rc[m', n'] * B_r[m', p']
    stage2: Y_rc[p', q']  = sum_{n'<256} Dt_rc[n', p'] * B_c[n', q']
    out[2p'+r, 2q'+c] = Y_rc[p', q']
with B_e[m', p'] = C'[2p', m'], B_o[m', p'] = C'[2p'+1, m'] (identical for both
stages since M == N). The host stages rows AND columns in the self-pairing
order [0..127, 255..128, 511..384, 256..383] so every fold is a plain
partition-aligned elementwise add/sub; the basis is stored in the same
permuted contraction order (contraction order is irrelevant to the matmul).

Per image: 32 bf16 matmuls x 256 free columns (8192 PE cycles, vs 12288 for
the fp32r single-fold variant). bf16 stationary loads use the HW fast-weight-
load path so LDWEIGHTS hides under the 256-column matmuls (fp32r LDW did not).
bf16 I/O halves DMA to 1 MB/image; the host applies the inverse permutation
and upcasts the bf16 result to f32.
"""

import sys
import numpy as np
import ml_dtypes

for _p in ("/opt/trn_rl_repo", "/opt/pypackages"):
    if _p not in sys.path:
        sys.path.append(_p)

import concourse.tile as tile  # noqa: E402
from concourse import bacc, mybir  # noqa: E402
from concourse.bass_utils import run_bass_kernel_spmd  # noqa: E402

N_CORES = 8
B_FULL = 128
S = 512
H = 256
BPC = B_FULL // N_CORES  # images per core

BF16 = mybir.dt.bfloat16

# Stored index -> original index, self-similar fold order (rows and columns).
PERM = np.concatenate(
    [
        np.arange(0, 128),
        np.arange(255, 127, -1),
        np.arange(511, 383, -1),
        np.arange(256, 384),
    ]
)
PERM256 = PERM[:256]


def _basis_np():
    """B_e/B_o [256 stored-contraction, 256 out] in the stored (permuted) order."""
    j = np.arange(S, dtype=np.float64)
    k = np.arange(S, dtype=np.float64)
    c = np.cos(np.pi * (2.0 * j[None, :] + 1.0) * k[:, None] / (2.0 * S))
    s = np.full(S, np.sqrt(2.0 / S))
    s[0] /= np.sqrt(2.0)
    C = c * s[:, None]  # C'[k, j]
    ET = C[0::2, :][:, PERM256].T.copy()  # [256 stored m', 256 p']
    OT = C[1::2, :][:, PERM256].T.copy()

    def to_tiles(M):  # [256, 256] -> [128, 2, 256]
        return np.ascontiguousarray(
            M.reshape(2, 128, 256).transpose(1, 0, 2)
        ).astype(ml_dtypes.bfloat16)

    return to_tiles(ET), to_tiles(OT)


def _build():
    nc = bacc.Bacc("TRN2", target_bir_lowering=False, debug=False)
    in_d = nc.dram_tensor("inp", [BPC, 128, 4, S], BF16, kind="ExternalInput").ap()
    et_d = nc.dram_tensor("et", [128, 2, H], BF16, kind="ExternalInput").ap()
    ot_d = nc.dram_tensor("ot", [128, 2, H], BF16, kind="ExternalInput").ap()
    out_d = nc.dram_tensor(
        "out", [BPC, 128, 2, 2, 2, H], BF16, kind="ExternalOutput"
    ).ap()

    with tile.TileContext(nc) as tc:
        with (
            tc.tile_pool(name="const", bufs=1) as cpool,
            tc.tile_pool(name="a", bufs=3) as apool,
            tc.tile_pool(name="row", bufs=4) as rpool,
            tc.tile_pool(name="quad", bufs=8) as qpool,
            tc.tile_pool(name="dt", bufs=4) as dtpool,
            tc.tile_pool(name="st", bufs=2) as stpool,
            tc.tile_pool(name="ps1", bufs=2, space="PSUM") as ps1pool,
            tc.tile_pool(name="ps2", bufs=2, space="PSUM") as ps2pool,
        ):
            et_sb = cpool.tile([128, 2, H], BF16)
            ot_sb = cpool.tile([128, 2, H], BF16)

            def emit_load(i):
                a = apool.tile([128, 4, S], BF16, tag="a", name=f"a_{i}")
                nc.sync.dma_start(a[:], in_d[i])
                return a

            def emit_folds(i, a):
                """Row folds then column folds; returns quad tiles (ee, eo, oe, oo)."""
                rowE = rpool.tile([128, 2, S], BF16, tag="row", name=f"re_{i}")
                rowO = rpool.tile([128, 2, S], BF16, tag="row", name=f"ro_{i}")
                nc.vector.tensor_add(rowE[:], a[:, 0:2, :], a[:, 2:4, :])
                nc.vector.tensor_sub(rowO[:], a[:, 0:2, :], a[:, 2:4, :])
                quads = {}
                for (qname, src, op_eng, is_add) in (
                    ("ee", rowE, nc.vector, True),
                    ("oe", rowO, nc.vector, True),
                    ("eo", rowE, nc.gpsimd, False),
                    ("oo", rowO, nc.gpsimd, False),
                ):
                    q = qpool.tile([128, 2, H], BF16, tag="quad", name=f"{qname}_{i}")
                    f = op_eng.tensor_add if is_add else op_eng.tensor_sub
                    f(q[:], src[:, :, 0:H], src[:, :, H:S])
                    quads[qname] = q
                return quads

            a0 = emit_load(0)
            nc.sync.dma_start(et_sb[:], et_d)
            nc.sync.dma_start(ot_sb[:], ot_d)
            a1 = emit_load(1)
            pend = {0: a0, 1: a1}
            folds = emit_folds(0, pend.pop(0))

            bas = {"e": et_sb, "o": ot_sb}
            for i in range(BPC):
                quads = folds

                # ---- stage 1: Dt_rc[n', p'], psum pair tiles [128, q(2), ns(2), 256]
                ps1 = {}
                dts = {}
                for pi, pair in enumerate((("ee", "eo"), ("oe", "oo"))):
                    p1 = ps1pool.tile(
                        [128, 2, 2, H], mybir.dt.float32, tag="ps1", name=f"ps1_{i}_{pi}"
                    )
                    ps1[pair] = p1
                    for qi, qname in enumerate(pair):
                        r = qname[0]
                        for ns in range(2):
                            for t in range(2):
                                nc.tensor.matmul(
                                    p1[:, qi, ns, :],
                                    quads[qname][:, t, ns * 128 : (ns + 1) * 128],
                                    bas[r][:, t, :],
                                    start=(t == 0),
                                    stop=(t == 1),
                                )
                    # copy pair -> sbuf bf16 (stage-2 stationary input)
                    dt = dtpool.tile([128, 2, 2, H], BF16, tag="dt", name=f"dt_{i}_{pi}")
                    dts[pair] = dt
                    if pi == 0:
                        # prefetch + next folds between the two stage-1 halves
                        if i + 2 < BPC:
                            pend[i + 2] = emit_load(i + 2)
                        if i + 1 < BPC:
                            folds = emit_folds(i + 1, pend.pop(i + 1))
                    nc.scalar.copy(dt[:], p1[:])

                # ---- stage 2: Y_rc[p', q'], psum pair tiles [128, q(2), ps(2), 256]
                st = stpool.tile([128, 2, 2, 2, H], BF16, tag="st", name=f"st_{i}")
                for pi, pair in enumerate((("ee", "eo"), ("oe", "oo"))):
                    p2 = ps2pool.tile(
                        [128, 2, 2, H], mybir.dt.float32, tag="ps2", name=f"ps2_{i}_{pi}"
                    )
                    dt = dts[pair]
                    for qi, qname in enumerate(pair):
                        c = qname[1]
                        for ps in range(2):
                            for t2 in range(2):
                                nc.tensor.matmul(
                                    p2[:, qi, ps, :],
                                    dt[:, qi, t2, ps * 128 : (ps + 1) * 128],
                                    bas[c][:, t2, :],
                                    start=(t2 == 0),
                                    stop=(t2 == 1),
                                )
                    eng = nc.scalar if pi == 0 else nc.vector
                    cp = eng.copy if pi == 0 else eng.tensor_copy
                    cp(st[:, pi], p2[:])
                nc.sync.dma_start(out_d[i], st[:])
    nc.compile()
    return nc


_NC_CACHE = None


def _get_nc():
    global _NC_CACHE
    if _NC_CACHE is None:
        _NC_CACHE = _build()
    return _NC_CACHE


def run_sharded(img: np.ndarray, **spmd_kwargs):
    """img [128, 1, 512, 512] f32 -> (out [128, 1, 512, 512] f32, results)."""
    img = np.asarray(img, dtype=np.float32).reshape(B_FULL, S, S)
    # host staging: permute rows+cols into fold order, tile rows into 4 groups
    x = img[:, PERM, :][:, :, PERM]
    xt = np.ascontiguousarray(
        x.reshape(B_FULL, 4, 128, S).transpose(0, 2, 1, 3)
    ).astype(ml_dtypes.bfloat16)  # [B, 128, 4, 512]
    et, ot = _basis_np()
    nc = _get_nc()
    in_maps = [
        {"inp": xt[k * BPC : (k + 1) * BPC], "et": et, "ot": ot}
        for k in range(N_CORES)
    ]
    res = run_bass_kernel_spmd(nc, in_maps, core_ids=list(range(N_CORES)), **spmd_kwargs)
    O = np.empty((B_FULL, 128, 2, 2, 2, H), dtype=np.float32)
    for k in range(N_CORES):
        O[k * BPC : (k + 1) * BPC] = np.asarray(res.results[k]["out"], dtype=np.float32)
    # O[b, u, r, c, ps, q'] = Y_rc[ps*128+u, q'] -> out[b, 2*(ps*128+u)+r, 2*q'+c]
    out = O.transpose(0, 4, 1, 2, 5, 3).reshape(B_FULL, S, S)
    return np.ascontiguousarray(out).reshape(B_FULL, 1, S, S), res


def kernel(img: np.ndarray) -> np.ndarray:
    out, _ = run_sharded(img)
    return out


# revision 10
# speedup vs baseline: 1.5120x; 1.1274x over previous
"""2D DCT-II (separable) kernel for Trainium2, data-parallel over 8 NeuronCores.

Problem: img [128, 1, 512, 512] f32 -> out [128, 1, 512, 512] f32 with
    out[b,0] = scale * (Cp @ img[b,0] @ Cq^T),  Cp == Cq, scale rank-1 folded in:
    C'[k,j] = s_k * cos(pi*(2j+1)*k/1024),  s_k = sqrt(2/512) * (1/sqrt2 if k==0)
    out[b] = C' @ A @ C'^T

Scheme (all bf16; rel-err budget 2e-2 >> bf16's ~4e-3):
The DCT reflection symmetry C'[k, 511-j] = (-1)^k C'[k, j] folds BOTH
contractions from 512 to 256 ("quadrant folding"), halving PE work vs the
unfolded separable transform. The host stages rows AND columns in the
self-pairing order [0..127, 255..128, 511..384, 256..383] (pure permutation)
so every fold is partition/column-aligned, and stores the basis in the same
permuted contraction order (contraction order is free).

    rowE/rowO[m', c] = A[m', c] +/- A[pair(m'), c]        (DVE add / Pool sub)
    stage1 (per row-parity r, basis B_r):                  16 matmuls x 256
        psL[n', p'] = sum_m' rowX[m', n']      B_r[m', p']   (left col half)
        psR[n', p'] = sum_m' rowX[m', n'+256]  B_r[m', p']   (right col half)
    col fold fused into the psum->sbuf copy (DVE):
        dt_re = psL + psR, dt_ro = psL - psR               (bf16)
    stage2 (per quadrant rc, basis B_c):                   16 matmuls x 256
        Y_rc[p', q'] = sum_n' dt_rc[n', p'] B_c[n', q']
    out[2p'+r, 2q'+c] = Y_rc[p', q']                       (host un-permutes)

Per image: 32 bf16 matmuls x 256 free columns (8192 PE cycles at 2.4 GHz vs
12288 fp32r cycles for the single-fold baseline). bf16 stationary loads use
the fast-weight-load path so LDWEIGHTS hides under the matmuls. Stage 2 runs
one image behind stage 1 (software pipeline) so the fused fold-copies are
never on the PE critical path. bf16 I/O halves DMA to 1 MB/image.
"""

import sys
import numpy as np
import ml_dtypes

for _p in ("/opt/trn_rl_repo", "/opt/pypackages"):
    if _p not in sys.path:
        sys.path.append(_p)

import concourse.tile as tile  # noqa: E402
from concourse import bacc, mybir  # noqa: E402
from concourse.bass_utils import run_bass_kernel_spmd  # noqa: E402

N_CORES = 8
B_FULL = 128
S = 512
H = 256
BPC = B_FULL // N_CORES  # images per core

BF16 = mybir.dt.bfloat16

# Stored index -> original index, self-similar fold order (rows and columns).
PERM = np.concatenate(
    [
        np.arange(0, 128),
        np.arange(255, 127, -1),
        np.arange(511, 383, -1),
        np.arange(256, 384),
    ]
)
PERM256 = PERM[:256]


def _basis_np():
    """B_e/B_o [256 stored-contraction, 256 out] in the stored (permuted) order."""
    j = np.arange(S, dtype=np.float64)
    k = np.arange(S, dtype=np.float64)
    c = np.cos(np.pi * (2.0 * j[None, :] + 1.0) * k[:, None] / (2.0 * S))
    s = np.full(S, np.sqrt(2.0 / S))
    s[0] /= np.sqrt(2.0)
    C = c * s[:, None]  # C'[k, j]
    ET = C[0::2, :][:, PERM256].T.copy()  # [256 stored m', 256 p']
    OT = C[1::2, :][:, PERM256].T.copy()

    def to_tiles(M):  # [256, 256] -> [128, 2, 256]
        return np.ascontiguousarray(
            M.reshape(2, 128, 256).transpose(1, 0, 2)
        ).astype(ml_dtypes.bfloat16)

    return to_tiles(ET), to_tiles(OT)


def _build():
    nc = bacc.Bacc("TRN2", target_bir_lowering=False, debug=False)
    in_d = nc.dram_tensor("inp", [BPC, 128, 4, S], BF16, kind="ExternalInput").ap()
    et_d = nc.dram_tensor("et", [128, 2, H], BF16, kind="ExternalInput").ap()
    ot_d = nc.dram_tensor("ot", [128, 2, H], BF16, kind="ExternalInput").ap()
    out_d = nc.dram_tensor(
        "out", [BPC, 128, 2, 2, 2, H], BF16, kind="ExternalOutput"
    ).ap()

    with tile.TileContext(nc) as tc:
        with (
            tc.tile_pool(name="const", bufs=1) as cpool,
            tc.tile_pool(name="a", bufs=3) as apool,
            tc.tile_pool(name="row", bufs=4) as rpool,
            tc.tile_pool(name="dt", bufs=8) as dtpool,
            tc.tile_pool(name="st", bufs=2) as stpool,
            tc.tile_pool(name="ps1", bufs=4, space="PSUM") as ps1pool,
            tc.tile_pool(name="ps2", bufs=2, space="PSUM") as ps2pool,
        ):
            et_sb = cpool.tile([128, 2, H], BF16)
            ot_sb = cpool.tile([128, 2, H], BF16)
            bas = {"e": et_sb, "o": ot_sb}

            def emit_load(i):
                a = apool.tile([128, 4, S], BF16, tag="a", name=f"a_{i}")
                nc.sync.dma_start(a[:], in_d[i])
                return a

            def emit_folds(i, a):
                """rowE = top + bottom (DVE, 2x add), rowO = top - bottom (Pool)."""
                rowE = rpool.tile([128, 2, S], BF16, tag="row", name=f"re_{i}")
                rowO = rpool.tile([128, 2, S], BF16, tag="row", name=f"ro_{i}")
                nc.vector.tensor_add(rowE[:], a[:, 0:2, :], a[:, 2:4, :])
                nc.gpsimd.tensor_sub(rowO[:], a[:, 0:2, :], a[:, 2:4, :])
                return rowE, rowO

            a0 = emit_load(0)
            nc.sync.dma_start(et_sb[:], et_d)
            nc.sync.dma_start(ot_sb[:], ot_d)
            a1 = emit_load(1)
            pend = {0: a0, 1: a1}
            cur_folds = emit_folds(0, pend.pop(0))
            prev_dt = None

            for i in range(BPC + 1):
                nxt_dt = None
                if i < BPC:
                    rowX = {"e": cur_folds[0], "o": cur_folds[1]}
                    # ---- stage 1: psL/psR per row-parity, [128, ns(2), 256]
                    ps = {}
                    for r in ("e", "o"):
                        src, b = rowX[r], bas[r]
                        pl = ps1pool.tile(
                            [128, 2, H], mybir.dt.float32, tag="ps1", name=f"pl_{i}_{r}"
                        )
                        pr = ps1pool.tile(
                            [128, 2, H], mybir.dt.float32, tag="ps1", name=f"pr_{i}_{r}"
                        )
                        for half, pst in ((0, pl), (1, pr)):
                            for ns in range(2):
                                for t in range(2):
                                    o = half * H + ns * 128
                                    nc.tensor.matmul(
                                        pst[:, ns, :],
                                        src[:, t, o : o + 128],
                                        b[:, t, :],
                                        start=(t == 0),
                                        stop=(t == 1),
                                    )
                        ps[r] = (pl, pr)
                    # prefetch + next image's row folds
                    if i + 2 < BPC:
                        pend[i + 2] = emit_load(i + 2)
                    if i + 1 < BPC:
                        cur_folds = emit_folds(i + 1, pend.pop(i + 1))
                    # ---- fused col-fold copies: dt_re = psL+psR, dt_ro = psL-psR
                    # (dual-PSUM tensor_tensor is illegal, so stage psL to SBUF
                    # f32 on ACT, then DVE combines SBUF + PSUM.)
                    nxt_dt = {}
                    for r in ("e", "o"):
                        pl, pr = ps[r]
                        sl = dtpool.tile(
                            [128, 2, H], mybir.dt.float32, tag="sl", name=f"sl_{i}_{r}"
                        )
                        nc.scalar.copy(sl[:], pl[:])
                        da = dtpool.tile([128, 2, H], BF16, tag="dt", name=f"da_{i}_{r}")
                        ds = dtpool.tile([128, 2, H], BF16, tag="dt", name=f"ds_{i}_{r}")
                        nc.vector.tensor_add(da[:], sl[:], pr[:])
                        nc.vector.tensor_sub(ds[:], sl[:], pr[:])
                        nxt_dt[r] = (da, ds)

                if i >= 1:
                    # ---- stage 2 for image i-1: Y_rc[p', q']
                    j = i - 1
                    st = stpool.tile([128, 2, 2, 2, H], BF16, tag="st", name=f"st_{j}")
                    for pi, r in enumerate(("e", "o")):
                        da, ds = prev_dt[r]
                        p2 = ps2pool.tile(
                            [128, 2, 2, H], mybir.dt.float32, tag="ps2", name=f"p2_{j}_{r}"
                        )
                        for qi, (dtq, c) in enumerate(((da, "e"), (ds, "o"))):
                            for psl in range(2):
                                for t2 in range(2):
                                    nc.tensor.matmul(
                                        p2[:, qi, psl, :],
                                        dtq[:, t2, psl * 128 : (psl + 1) * 128],
                                        bas[c][:, t2, :],
                                        start=(t2 == 0),
                                        stop=(t2 == 1),
                                    )
                        nc.scalar.copy(st[:, pi], p2[:])
                    nc.sync.dma_start(out_d[j], st[:])
                prev_dt = nxt_dt
    nc.compile()
    return nc


_NC_CACHE = None


def _get_nc():
    global _NC_CACHE
    if _NC_CACHE is None:
        _NC_CACHE = _build()
    return _NC_CACHE


def run_sharded(img: np.ndarray, **spmd_kwargs):
    """img [128, 1, 512, 512] f32 -> (out [128, 1, 512, 512] f32, results)."""
    img = np.asarray(img, dtype=np.float32).reshape(B_FULL, S, S)
    # host staging: permute rows+cols into fold order, tile rows into 4 groups
    x = img[:, PERM, :][:, :, PERM]
    xt = np.ascontiguousarray(
        x.reshape(B_FULL, 4, 128, S).transpose(0, 2, 1, 3)
    ).astype(ml_dtypes.bfloat16)  # [B, 128, 4, 512]
    et, ot = _basis_np()
    nc = _get_nc()
    in_maps = [
        {"inp": xt[k * BPC : (k + 1) * BPC], "et": et, "ot": ot}
        for k in range(N_CORES)
    ]
    res = run_bass_kernel_spmd(nc, in_maps, core_ids=list(range(N_CORES)), **spmd_kwargs)
    O = np.empty((B_FULL, 128, 2, 2, 2, H), dtype=np.float32)
    for k in range(N_CORES):
        O[k * BPC : (k + 1) * BPC] = np.asarray(res.results[k]["out"], dtype=np.float32)
    # O[b, u, r, c, ps, q'] = Y_rc[ps*128+u, q'] -> out[b, 2*(ps*128+u)+r, 2*q'+c]
    out = O.transpose(0, 4, 1, 2, 5, 3).reshape(B_FULL, S, S)
    return np.ascontiguousarray(out).reshape(B_FULL, 1, S, S), res


def kernel(img: np.ndarray) -> np.ndarray:
    out, _ = run_sharded(img)
    return out


# revision 14
# speedup vs baseline: 1.5952x; 1.0550x over previous
"""2D DCT-II (separable) kernel for Trainium2, data-parallel over 8 NeuronCores.

Problem: img [128, 1, 512, 512] f32 -> out [128, 1, 512, 512] f32 with
    out[b,0] = scale * (Cp @ img[b,0] @ Cq^T),  Cp == Cq, scale rank-1 folded in:
    C'[k,j] = s_k * cos(pi*(2j+1)*k/1024),  s_k = sqrt(2/512) * (1/sqrt2 if k==0)
    out[b] = C' @ A @ C'^T

Scheme (all bf16; rel-err budget 2e-2 >> bf16's ~4e-3):
The DCT reflection symmetry C'[k, 511-j] = (-1)^k C'[k, j] folds BOTH
contractions from 512 to 256 ("quadrant folding"), halving PE work vs the
unfolded separable transform. The host stages rows AND columns in the
self-pairing order [0..127, 255..128, 511..384, 256..383] (pure permutation)
so every fold is partition/column-aligned, and stores the basis in the same
permuted contraction order (contraction order is free).

    rowE/rowO[m', c] = A[m', c] +/- A[pair(m'), c]        (DVE add / Pool sub)
    stage1 (per row-parity r, basis B_r):                  16 matmuls x 256
        psL[n', p'] = sum_m' rowX[m', n']      B_r[m', p']   (left col half)
        psR[n', p'] = sum_m' rowX[m', n'+256]  B_r[m', p']   (right col half)
    col fold fused into the psum->sbuf copy (DVE):
        dt_re = psL + psR, dt_ro = psL - psR               (bf16)
    stage2 (per quadrant rc, basis B_c):                   16 matmuls x 256
        Y_rc[p', q'] = sum_n' dt_rc[n', p'] B_c[n', q']
    out[2p'+r, 2q'+c] = Y_rc[p', q']                       (host un-permutes)

Per image: 32 bf16 matmuls x 256 free columns (8192 PE cycles at 2.4 GHz vs
12288 fp32r cycles for the single-fold baseline). bf16 stationary loads use
the fast-weight-load path so LDWEIGHTS hides under the matmuls. Stage 2 runs
one image behind stage 1 (software pipeline) so the fused fold-copies are
never on the PE critical path. bf16 I/O halves DMA to 1 MB/image.
"""

import sys
import numpy as np
import ml_dtypes

for _p in ("/opt/trn_rl_repo", "/opt/pypackages"):
    if _p not in sys.path:
        sys.path.append(_p)

import concourse.tile as tile  # noqa: E402
from concourse import bacc, mybir  # noqa: E402
from concourse.bass_utils import run_bass_kernel_spmd  # noqa: E402

N_CORES = 8
B_FULL = 128
S = 512
H = 256
BPC = B_FULL // N_CORES  # images per core

BF16 = mybir.dt.bfloat16

# Stored index -> original index, self-similar fold order (rows and columns).
PERM = np.concatenate(
    [
        np.arange(0, 128),
        np.arange(255, 127, -1),
        np.arange(511, 383, -1),
        np.arange(256, 384),
    ]
)
PERM256 = PERM[:256]


def _basis_np():
    """B_e/B_o [256 stored-contraction, 256 out] in the stored (permuted) order."""
    j = np.arange(S, dtype=np.float64)
    k = np.arange(S, dtype=np.float64)
    c = np.cos(np.pi * (2.0 * j[None, :] + 1.0) * k[:, None] / (2.0 * S))
    s = np.full(S, np.sqrt(2.0 / S))
    s[0] /= np.sqrt(2.0)
    C = c * s[:, None]  # C'[k, j]
    ET = C[0::2, :][:, PERM256].T.copy()  # [256 stored m', 256 p']
    OT = C[1::2, :][:, PERM256].T.copy()

    def to_tiles(M):  # [256, 256] -> [128, 2, 256]
        return np.ascontiguousarray(
            M.reshape(2, 128, 256).transpose(1, 0, 2)
        ).astype(ml_dtypes.bfloat16)

    return to_tiles(ET), to_tiles(OT)


def _build():
    nc = bacc.Bacc("TRN2", target_bir_lowering=False, debug=False)
    in_d = nc.dram_tensor("inp", [BPC, 128, 4, S], BF16, kind="ExternalInput").ap()
    et_d = nc.dram_tensor("et", [128, 2, H], BF16, kind="ExternalInput").ap()
    ot_d = nc.dram_tensor("ot", [128, 2, H], BF16, kind="ExternalInput").ap()
    out_d = nc.dram_tensor(
        "out", [BPC, 128, 2, 2, 2, H], BF16, kind="ExternalOutput"
    ).ap()

    with tile.TileContext(nc) as tc:
        with (
            tc.tile_pool(name="const", bufs=1) as cpool,
            tc.tile_pool(name="a", bufs=4) as apool,
            tc.tile_pool(name="a0", bufs=1) as a0pool,
            tc.tile_pool(name="row", bufs=4) as rpool,
            tc.tile_pool(name="dt", bufs=8) as dtpool,
            tc.tile_pool(name="st", bufs=3) as stpool,
            tc.tile_pool(name="ps1", bufs=4, space="PSUM") as ps1pool,
            tc.tile_pool(name="ps2", bufs=2, space="PSUM") as ps2pool,
        ):
            et_sb = cpool.tile([128, 2, H], BF16)
            ot_sb = cpool.tile([128, 2, H], BF16)
            bas = {"e": et_sb, "o": ot_sb}

            def emit_load(i):
                a = apool.tile([128, 4, S], BF16, tag="a", name=f"a_{i}")
                nc.sync.dma_start(a[:], in_d[i])
                return a

            def emit_folds(i, a):
                """rowE = top + bottom (DVE, 2x add), rowO = top - bottom (Pool)."""
                rowE = rpool.tile([128, 2, S], BF16, tag="row", name=f"re_{i}")
                rowO = rpool.tile([128, 2, S], BF16, tag="row", name=f"ro_{i}")
                nc.vector.tensor_add(rowE[:], a[:, 0:2, :], a[:, 2:4, :])
                nc.gpsimd.tensor_sub(rowO[:], a[:, 0:2, :], a[:, 2:4, :])
                return rowE, rowO

            # ---- fast-path startup: image 0 loads as two fold-pair halves so
            # its folds (and first matmuls) start as early as possible; the
            # basis rides the idle Scalar DMA queue in parallel.
            a0A = a0pool.tile([128, 2, S], BF16, name="a0A")
            a0B = a0pool.tile([128, 2, S], BF16, name="a0B")
            nc.sync.dma_start(a0A[:], in_d[0, :, 0:3:2, :])  # groups 0, 2
            nc.scalar.dma_start(et_sb[:], et_d)
            nc.scalar.dma_start(ot_sb[:], ot_d)
            nc.sync.dma_start(a0B[:], in_d[0, :, 1:4:2, :])  # groups 1, 3
            a1 = emit_load(1)
            # PE warm-up on the basis while image 0 is still in flight: keeps
            # the HAM activity window busy so real matmuls start at 2.4 GHz.
            wu = ps2pool.tile([128, 2, 2, H], mybir.dt.float32, tag="ps2", name="warmup")
            for k in range(14):
                nc.tensor.matmul(
                    wu[:, k % 2, 0, :],
                    et_sb[:, 0, 0:128],
                    et_sb[:, k % 2, :],
                    start=True,
                    stop=True,
                )
            # image-0 folds, split per row-tile so t=0 follows the first DMA
            rowE0 = rpool.tile([128, 2, S], BF16, tag="row", name="re_0")
            rowO0 = rpool.tile([128, 2, S], BF16, tag="row", name="ro_0")
            nc.vector.tensor_add(rowE0[:, 0, :], a0A[:, 0, :], a0A[:, 1, :])
            nc.gpsimd.tensor_sub(rowO0[:, 0, :], a0A[:, 0, :], a0A[:, 1, :])
            nc.vector.tensor_add(rowE0[:, 1, :], a0B[:, 0, :], a0B[:, 1, :])
            nc.gpsimd.tensor_sub(rowO0[:, 1, :], a0B[:, 0, :], a0B[:, 1, :])
            pend = {1: a1}
            cur_folds = (rowE0, rowO0)
            prev_dt = None

            for i in range(BPC + 1):
                nxt_dt = None
                if i < BPC:
                    rowX = {"e": cur_folds[0], "o": cur_folds[1]}
                    # ---- stage 1: psL/psR per row-parity, [128, ns(2), 256]
                    ps = {}
                    for r in ("e", "o"):
                        src, b = rowX[r], bas[r]
                        pl = ps1pool.tile(
                            [128, 2, H], mybir.dt.float32, tag="ps1", name=f"pl_{i}_{r}"
                        )
                        pr = ps1pool.tile(
                            [128, 2, H], mybir.dt.float32, tag="ps1", name=f"pr_{i}_{r}"
                        )
                        for half, pst in ((0, pl), (1, pr)):
                            for ns in range(2):
                                for t in range(2):
                                    o = half * H + ns * 128
                                    nc.tensor.matmul(
                                        pst[:, ns, :],
                                        src[:, t, o : o + 128],
                                        b[:, t, :],
                                        start=(t == 0),
                                        stop=(t == 1),
                                    )
                        ps[r] = (pl, pr)
                    # prefetch + next image's row folds
                    if i + 2 < BPC:
                        pend[i + 2] = emit_load(i + 2)
                    if i + 1 < BPC:
                        cur_folds = emit_folds(i + 1, pend.pop(i + 1))
                    # ---- fused col-fold copies: dt_re = psL+psR, dt_ro = psL-psR
                    # (dual-PSUM tensor_tensor is illegal, so stage psL to SBUF
                    # f32 on ACT, then DVE combines SBUF + PSUM.)
                    nxt_dt = {}
                    for r in ("e", "o"):
                        pl, pr = ps[r]
                        sl = dtpool.tile(
                            [128, 2, H], mybir.dt.float32, tag="sl", name=f"sl_{i}_{r}"
                        )
                        nc.scalar.copy(sl[:], pl[:])
                        da = dtpool.tile([128, 2, H], BF16, tag="dt", name=f"da_{i}_{r}")
                        ds = dtpool.tile([128, 2, H], BF16, tag="dt", name=f"ds_{i}_{r}")
                        nc.vector.tensor_add(da[:], sl[:], pr[:])
                        nc.vector.tensor_sub(ds[:], sl[:], pr[:])
                        nxt_dt[r] = (da, ds)

                if i >= 1:
                    # ---- stage 2 for image i-1: Y_rc[p', q']
                    j = i - 1
                    st = stpool.tile([128, 2, 2, 2, H], BF16, tag="st", name=f"st_{j}")
                    for pi, r in enumerate(("e", "o")):
                        da, ds = prev_dt[r]
                        p2 = ps2pool.tile(
                            [128, 2, 2, H], mybir.dt.float32, tag="ps2", name=f"p2_{j}_{r}"
                        )
                        for qi, (dtq, c) in enumerate(((da, "e"), (ds, "o"))):
                            for psl in range(2):
                                for t2 in range(2):
                                    nc.tensor.matmul(
                                        p2[:, qi, psl, :],
                                        dtq[:, t2, psl * 128 : (psl + 1) * 128],
                                        bas[c][:, t2, :],
                                        start=(t2 == 0),
                                        stop=(t2 == 1),
                                    )
                        if j == BPC - 1 and pi == 1:
                            # tail drain: parallelize the final copy + DMA
                            nc.vector.tensor_copy(st[:, pi], p2[:])
                            nc.sync.dma_start(out_d[j, :, pi], st[:, pi])
                        else:
                            nc.scalar.copy(st[:, pi], p2[:])
                            if j == BPC - 1:
                                nc.scalar.dma_start(out_d[j, :, pi], st[:, pi])
                    if j < BPC - 1:
                        nc.sync.dma_start(out_d[j], st[:])
                prev_dt = nxt_dt
    nc.compile()
    return nc


_NC_CACHE = None


def _get_nc():
    global _NC_CACHE
    if _NC_CACHE is None:
        _NC_CACHE = _build()
    return _NC_CACHE


def run_sharded(img: np.ndarray, **spmd_kwargs):
    """img [128, 1, 512, 512] f32 -> (out [128, 1, 512, 512] f32, results)."""
    img = np.asarray(img, dtype=np.float32).reshape(B_FULL, S, S)
    # host staging: permute rows+cols into fold order, tile rows into 4 groups
    x = img[:, PERM, :][:, :, PERM]
    xt = np.ascontiguousarray(
        x.reshape(B_FULL, 4, 128, S).transpose(0, 2, 1, 3)
    ).astype(ml_dtypes.bfloat16)  # [B, 128, 4, 512]
    et, ot = _basis_np()
    nc = _get_nc()
    in_maps = [
        {"inp": xt[k * BPC : (k + 1) * BPC], "et": et, "ot": ot}
        for k in range(N_CORES)
    ]
    res = run_bass_kernel_spmd(nc, in_maps, core_ids=list(range(N_CORES)), **spmd_kwargs)
    O = np.empty((B_FULL, 128, 2, 2, 2, H), dtype=np.float32)
    for k in range(N_CORES):
        O[k * BPC : (k + 1) * BPC] = np.asarray(res.results[k]["out"], dtype=np.float32)
    # O[b, u, r, c, ps, q'] = Y_rc[ps*128+u, q'] -> out[b, 2*(ps*128+u)+r, 2*q'+c]
    out = O.transpose(0, 4, 1, 2, 5, 3).reshape(B_FULL, S, S)
    return np.ascontiguousarray(out).reshape(B_FULL, 1, S, S), res


def kernel(img: np.ndarray) -> np.ndarray:
    out, _ = run_sharded(img)
    return out


# revision 18
# speedup vs baseline: 1.6038x; 1.0054x over previous
"""2D DCT-II (separable) kernel for Trainium2, data-parallel over 8 NeuronCores.

Problem: img [128, 1, 512, 512] f32 -> out [128, 1, 512, 512] f32 with
    out[b,0] = scale * (Cp @ img[b,0] @ Cq^T),  Cp == Cq, scale rank-1 folded in:
    C'[k,j] = s_k * cos(pi*(2j+1)*k/1024),  s_k = sqrt(2/512) * (1/sqrt2 if k==0)
    out[b] = C' @ A @ C'^T

Scheme (all bf16; rel-err budget 2e-2 >> bf16's ~4e-3):
The DCT reflection symmetry C'[k, 511-j] = (-1)^k C'[k, j] folds BOTH
contractions from 512 to 256 ("quadrant folding"), halving PE work vs the
unfolded separable transform. The host stages rows AND columns in the
self-pairing order [0..127, 255..128, 511..384, 256..383] (pure permutation)
so every fold is partition/column-aligned, and stores the basis in the same
permuted contraction order (contraction order is free).

    rowE/rowO[m', c] = A[m', c] +/- A[pair(m'), c]        (DVE add / Pool sub)
    stage1 (per row-parity r, basis B_r):                  16 matmuls x 256
        psL[n', p'] = sum_m' rowX[m', n']      B_r[m', p']   (left col half)
        psR[n', p'] = sum_m' rowX[m', n'+256]  B_r[m', p']   (right col half)
    col fold fused into the psum->sbuf copy (DVE):
        dt_re = psL + psR, dt_ro = psL - psR               (bf16)
    stage2 (per quadrant rc, basis B_c):                   16 matmuls x 256
        Y_rc[p', q'] = sum_n' dt_rc[n', p'] B_c[n', q']
    out[2p'+r, 2q'+c] = Y_rc[p', q']                       (host un-permutes)

Per image: 32 bf16 matmuls x 256 free columns (8192 PE cycles at 2.4 GHz vs
12288 fp32r cycles for the single-fold baseline). bf16 stationary loads use
the fast-weight-load path so LDWEIGHTS hides under the matmuls. Stage 2 runs
one image behind stage 1 (software pipeline) so the fused fold-copies are
never on the PE critical path. bf16 I/O halves DMA to 1 MB/image.
"""

import sys
import numpy as np
import ml_dtypes

for _p in ("/opt/trn_rl_repo", "/opt/pypackages"):
    if _p not in sys.path:
        sys.path.append(_p)

import concourse.tile as tile  # noqa: E402
from concourse import bacc, mybir  # noqa: E402
from concourse.bass_utils import run_bass_kernel_spmd  # noqa: E402

N_CORES = 8
B_FULL = 128
S = 512
H = 256
BPC = B_FULL // N_CORES  # images per core

BF16 = mybir.dt.bfloat16

# Stored index -> original index, self-similar fold order (rows and columns).
PERM = np.concatenate(
    [
        np.arange(0, 128),
        np.arange(255, 127, -1),
        np.arange(511, 383, -1),
        np.arange(256, 384),
    ]
)
PERM256 = PERM[:256]


def _basis_np():
    """B_e/B_o [256 stored-contraction, 256 out] in the stored (permuted) order."""
    j = np.arange(S, dtype=np.float64)
    k = np.arange(S, dtype=np.float64)
    c = np.cos(np.pi * (2.0 * j[None, :] + 1.0) * k[:, None] / (2.0 * S))
    s = np.full(S, np.sqrt(2.0 / S))
    s[0] /= np.sqrt(2.0)
    C = c * s[:, None]  # C'[k, j]
    ET = C[0::2, :][:, PERM256].T.copy()  # [256 stored m', 256 p']
    OT = C[1::2, :][:, PERM256].T.copy()

    def to_tiles(M):  # [256, 256] -> [128, 2, 256]
        return np.ascontiguousarray(
            M.reshape(2, 128, 256).transpose(1, 0, 2)
        ).astype(ml_dtypes.bfloat16)

    return to_tiles(ET), to_tiles(OT)


def _build():
    nc = bacc.Bacc("TRN2", target_bir_lowering=False, debug=False)
    in_d = nc.dram_tensor("inp", [BPC, 128, 4, S], BF16, kind="ExternalInput").ap()
    et_d = nc.dram_tensor("et", [128, 2, H], BF16, kind="ExternalInput").ap()
    ot_d = nc.dram_tensor("ot", [128, 2, H], BF16, kind="ExternalInput").ap()
    out_d = nc.dram_tensor(
        "out", [BPC, 128, 2, 2, 2, H], BF16, kind="ExternalOutput"
    ).ap()

    with tile.TileContext(nc) as tc:
        with (
            tc.tile_pool(name="const", bufs=1) as cpool,
            tc.tile_pool(name="a", bufs=4) as apool,
            tc.tile_pool(name="a0", bufs=1) as a0pool,
            tc.tile_pool(name="row", bufs=4) as rpool,
            tc.tile_pool(name="dt", bufs=8) as dtpool,
            tc.tile_pool(name="st", bufs=3) as stpool,
            tc.tile_pool(name="ps1", bufs=4, space="PSUM") as ps1pool,
            tc.tile_pool(name="ps2", bufs=2, space="PSUM") as ps2pool,
        ):
            et_sb = cpool.tile([128, 2, H], BF16)
            ot_sb = cpool.tile([128, 2, H], BF16)
            bas = {"e": et_sb, "o": ot_sb}
            # PE warm-up on a never-written tile (values irrelevant, results
            # unread): no data dependency, so the PE is busy from ucode-load
            # time and the HAM clock-gate is at 2.4 GHz for the real matmuls.
            junk = cpool.tile([128, 2, H], BF16)
            nc.gpsimd.memset(junk[:], 0)

            def emit_load(i):
                a = apool.tile([128, 4, S], BF16, tag="a", name=f"a_{i}")
                nc.sync.dma_start(a[:], in_d[i])
                return a

            def emit_folds(i, a):
                """rowE = top + bottom (DVE, 2x add), rowO = top - bottom (Pool)."""
                rowE = rpool.tile([128, 2, S], BF16, tag="row", name=f"re_{i}")
                rowO = rpool.tile([128, 2, S], BF16, tag="row", name=f"ro_{i}")
                nc.vector.tensor_add(rowE[:], a[:, 0:2, :], a[:, 2:4, :])
                nc.gpsimd.tensor_sub(rowO[:], a[:, 0:2, :], a[:, 2:4, :])
                return rowE, rowO

            wu = ps2pool.tile([128, 2, 2, H], mybir.dt.float32, tag="ps2", name="warmup")
            for k in range(16):
                nc.tensor.matmul(
                    wu[:, k % 2, 0, :],
                    junk[:, 0, 0:128],
                    junk[:, k % 2, :],
                    start=True,
                    stop=True,
                )
            # ---- fast-path startup: image 0 loads as two fold-pair halves so
            # its folds (and first matmuls) start as early as possible; the
            # basis rides the idle Scalar DMA queue in parallel.
            a0A = a0pool.tile([128, 2, S], BF16, name="a0A")
            a0B = a0pool.tile([128, 2, S], BF16, name="a0B")
            nc.sync.dma_start(a0A[:], in_d[0, :, 0:3:2, :])  # groups 0, 2
            nc.scalar.dma_start(et_sb[:], et_d)
            nc.scalar.dma_start(ot_sb[:], ot_d)
            nc.sync.dma_start(a0B[:], in_d[0, :, 1:4:2, :])  # groups 1, 3
            a1 = emit_load(1)
            # image-0 folds, split per row-tile so t=0 follows the first DMA
            rowE0 = rpool.tile([128, 2, S], BF16, tag="row", name="re_0")
            rowO0 = rpool.tile([128, 2, S], BF16, tag="row", name="ro_0")
            nc.vector.tensor_add(rowE0[:, 0, :], a0A[:, 0, :], a0A[:, 1, :])
            nc.gpsimd.tensor_sub(rowO0[:, 0, :], a0A[:, 0, :], a0A[:, 1, :])
            nc.vector.tensor_add(rowE0[:, 1, :], a0B[:, 0, :], a0B[:, 1, :])
            nc.gpsimd.tensor_sub(rowO0[:, 1, :], a0B[:, 0, :], a0B[:, 1, :])
            pend = {1: a1}
            cur_folds = (rowE0, rowO0)
            prev_dt = None

            for i in range(BPC + 1):
                nxt_dt = None
                if i < BPC:
                    rowX = {"e": cur_folds[0], "o": cur_folds[1]}
                    # ---- stage 1: psL/psR per row-parity, [128, ns(2), 256]
                    ps = {}
                    for r in ("e", "o"):
                        src, b = rowX[r], bas[r]
                        pl = ps1pool.tile(
                            [128, 2, H], mybir.dt.float32, tag="ps1", name=f"pl_{i}_{r}"
                        )
                        pr = ps1pool.tile(
                            [128, 2, H], mybir.dt.float32, tag="ps1", name=f"pr_{i}_{r}"
                        )
                        for half, pst in ((0, pl), (1, pr)):
                            for ns in range(2):
                                for t in range(2):
                                    o = half * H + ns * 128
                                    nc.tensor.matmul(
                                        pst[:, ns, :],
                                        src[:, t, o : o + 128],
                                        b[:, t, :],
                                        start=(t == 0),
                                        stop=(t == 1),
                                    )
                        ps[r] = (pl, pr)
                    # prefetch + next image's row folds
                    if i + 2 < BPC:
                        pend[i + 2] = emit_load(i + 2)
                    if i + 1 < BPC:
                        cur_folds = emit_folds(i + 1, pend.pop(i + 1))
                    # ---- fused col-fold copies: dt_re = psL+psR, dt_ro = psL-psR
                    # (dual-PSUM tensor_tensor is illegal, so stage psL to SBUF
                    # f32 on ACT, then DVE combines SBUF + PSUM.)
                    nxt_dt = {}
                    for r in ("e", "o"):
                        pl, pr = ps[r]
                        sl = dtpool.tile(
                            [128, 2, H], mybir.dt.float32, tag="sl", name=f"sl_{i}_{r}"
                        )
                        nc.scalar.copy(sl[:], pl[:])
                        da = dtpool.tile([128, 2, H], BF16, tag="dt", name=f"da_{i}_{r}")
                        ds = dtpool.tile([128, 2, H], BF16, tag="dt", name=f"ds_{i}_{r}")
                        nc.vector.tensor_add(da[:], sl[:], pr[:])
                        nc.vector.tensor_sub(ds[:], sl[:], pr[:])
                        nxt_dt[r] = (da, ds)

                if i >= 1:
                    # ---- stage 2 for image i-1: Y_rc[p', q']
                    j = i - 1
                    st = stpool.tile([128, 2, 2, 2, H], BF16, tag="st", name=f"st_{j}")
                    for pi, r in enumerate(("e", "o")):
                        da, ds = prev_dt[r]
                        p2 = ps2pool.tile(
                            [128, 2, 2, H], mybir.dt.float32, tag="ps2", name=f"p2_{j}_{r}"
                        )
                        for qi, (dtq, c) in enumerate(((da, "e"), (ds, "o"))):
                            for psl in range(2):
                                for t2 in range(2):
                                    nc.tensor.matmul(
                                        p2[:, qi, psl, :],
                                        dtq[:, t2, psl * 128 : (psl + 1) * 128],
                                        bas[c][:, t2, :],
                                        start=(t2 == 0),
                                        stop=(t2 == 1),
                                    )
                            if j == BPC - 1:
                                # tail drain: copy each quarter right after its
                                # matmuls, split across ACT/DVE, DMA per half
                                cp = nc.scalar.copy if pi == 0 else nc.vector.tensor_copy
                                cp(st[:, pi, qi], p2[:, qi])
                        if j == BPC - 1:
                            eng = nc.scalar if pi == 0 else nc.sync
                            eng.dma_start(out_d[j, :, pi], st[:, pi])
                        else:
                            nc.scalar.copy(st[:, pi], p2[:])
                    if j < BPC - 1:
                        nc.sync.dma_start(out_d[j], st[:])
                prev_dt = nxt_dt
    nc.compile()
    return nc


_NC_CACHE = None


def _get_nc():
    global _NC_CACHE
    if _NC_CACHE is None:
        _NC_CACHE = _build()
    return _NC_CACHE


def run_sharded(img: np.ndarray, **spmd_kwargs):
    """img [128, 1, 512, 512] f32 -> (out [128, 1, 512, 512] f32, results)."""
    img = np.asarray(img, dtype=np.float32).reshape(B_FULL, S, S)
    # host staging: permute rows+cols into fold order, tile rows into 4 groups
    x = img[:, PERM, :][:, :, PERM]
    xt = np.ascontiguousarray(
        x.reshape(B_FULL, 4, 128, S).transpose(0, 2, 1, 3)
    ).astype(ml_dtypes.bfloat16)  # [B, 128, 4, 512]
    et, ot = _basis_np()
    nc = _get_nc()
    in_maps = [
        {"inp": xt[k * BPC : (k + 1) * BPC], "et": et, "ot": ot}
        for k in range(N_CORES)
    ]
    res = run_bass_kernel_spmd(nc, in_maps, core_ids=list(range(N_CORES)), **spmd_kwargs)
    O = np.empty((B_FULL, 128, 2, 2, 2, H), dtype=np.float32)
    for k in range(N_CORES):
        O[k * BPC : (k + 1) * BPC] = np.asarray(res.results[k]["out"], dtype=np.float32)
    # O[b, u, r, c, ps, q'] = Y_rc[ps*128+u, q'] -> out[b, 2*(ps*128+u)+r, 2*q'+c]
    out = O.transpose(0, 4, 1, 2, 5, 3).reshape(B_FULL, S, S)
    return np.ascontiguousarray(out).reshape(B_FULL, 1, S, S), res


def kernel(img: np.ndarray) -> np.ndarray:
    out, _ = run_sharded(img)
    return out


# revision 21
# speedup vs baseline: 1.6110x; 1.0045x over previous
"""2D DCT-II (separable) kernel for Trainium2, data-parallel over 8 NeuronCores.

Problem: img [128, 1, 512, 512] f32 -> out [128, 1, 512, 512] f32 with
    out[b,0] = scale * (Cp @ img[b,0] @ Cq^T),  Cp == Cq, scale rank-1 folded in:
    C'[k,j] = s_k * cos(pi*(2j+1)*k/1024),  s_k = sqrt(2/512) * (1/sqrt2 if k==0)
    out[b] = C' @ A @ C'^T

Scheme (all bf16; rel-err budget 2e-2 >> bf16's ~4e-3):
The DCT reflection symmetry C'[k, 511-j] = (-1)^k C'[k, j] folds BOTH
contractions from 512 to 256 ("quadrant folding"), halving PE work vs the
unfolded separable transform. The host stages rows AND columns in the
self-pairing order [0..127, 255..128, 511..384, 256..383] (pure permutation)
so every fold is partition/column-aligned, and stores the basis in the same
permuted contraction order (contraction order is free).

    rowE/rowO[m', c] = A[m', c] +/- A[pair(m'), c]        (DVE add / Pool sub)
    stage1 (per row-parity r, basis B_r):                  16 matmuls x 256
        psL[n', p'] = sum_m' rowX[m', n']      B_r[m', p']   (left col half)
        psR[n', p'] = sum_m' rowX[m', n'+256]  B_r[m', p']   (right col half)
    col fold fused into the psum->sbuf copy (DVE):
        dt_re = psL + psR, dt_ro = psL - psR               (bf16)
    stage2 (per quadrant rc, basis B_c):                   16 matmuls x 256
        Y_rc[p', q'] = sum_n' dt_rc[n', p'] B_c[n', q']
    out[2p'+r, 2q'+c] = Y_rc[p', q']                       (host un-permutes)

Per image: 32 bf16 matmuls x 256 free columns (8192 PE cycles at 2.4 GHz vs
12288 fp32r cycles for the single-fold baseline). bf16 stationary loads use
the fast-weight-load path so LDWEIGHTS hides under the matmuls. Stage 2 runs
one image behind stage 1 (software pipeline) so the fused fold-copies are
never on the PE critical path. bf16 I/O halves DMA to 1 MB/image.
"""

import sys
import numpy as np
import ml_dtypes

for _p in ("/opt/trn_rl_repo", "/opt/pypackages"):
    if _p not in sys.path:
        sys.path.append(_p)

import concourse.tile as tile  # noqa: E402
from concourse import bacc, mybir  # noqa: E402
from concourse.bass_utils import run_bass_kernel_spmd  # noqa: E402

N_CORES = 8
B_FULL = 128
S = 512
H = 256
BPC = B_FULL // N_CORES  # images per core

BF16 = mybir.dt.bfloat16

# Stored index -> original index, self-similar fold order (rows and columns).
PERM = np.concatenate(
    [
        np.arange(0, 128),
        np.arange(255, 127, -1),
        np.arange(511, 383, -1),
        np.arange(256, 384),
    ]
)
PERM256 = PERM[:256]


def _basis_np():
    """B_e/B_o [256 stored-contraction, 256 out] in the stored (permuted) order."""
    j = np.arange(S, dtype=np.float64)
    k = np.arange(S, dtype=np.float64)
    c = np.cos(np.pi * (2.0 * j[None, :] + 1.0) * k[:, None] / (2.0 * S))
    s = np.full(S, np.sqrt(2.0 / S))
    s[0] /= np.sqrt(2.0)
    C = c * s[:, None]  # C'[k, j]
    ET = C[0::2, :][:, PERM256].T.copy()  # [256 stored m', 256 p']
    OT = C[1::2, :][:, PERM256].T.copy()

    def to_tiles(M):  # [256, 256] -> [128, 2, 256]
        return np.ascontiguousarray(
            M.reshape(2, 128, 256).transpose(1, 0, 2)
        ).astype(ml_dtypes.bfloat16)

    return to_tiles(ET), to_tiles(OT)


def _build():
    nc = bacc.Bacc("TRN2", target_bir_lowering=False, debug=False)
    in_d = nc.dram_tensor("inp", [BPC, 128, 4, S], BF16, kind="ExternalInput").ap()
    et_d = nc.dram_tensor("et", [128, 2, H], BF16, kind="ExternalInput").ap()
    ot_d = nc.dram_tensor("ot", [128, 2, H], BF16, kind="ExternalInput").ap()
    out_d = nc.dram_tensor(
        "out", [BPC, 128, 2, 2, 2, H], BF16, kind="ExternalOutput"
    ).ap()

    with tile.TileContext(nc) as tc:
        with (
            tc.tile_pool(name="const", bufs=1) as cpool,
            tc.tile_pool(name="a", bufs=4) as apool,
            tc.tile_pool(name="a0", bufs=1) as a0pool,
            tc.tile_pool(name="row", bufs=4) as rpool,
            tc.tile_pool(name="dt", bufs=8) as dtpool,
            tc.tile_pool(name="slp", bufs=4) as slpool,
            tc.tile_pool(name="st", bufs=3) as stpool,
            tc.tile_pool(name="ps1", bufs=4, space="PSUM") as ps1pool,
            tc.tile_pool(name="ps2", bufs=2, space="PSUM") as ps2pool,
        ):
            et_sb = cpool.tile([128, 2, H], BF16)
            ot_sb = cpool.tile([128, 2, H], BF16)
            bas = {"e": et_sb, "o": ot_sb}
            # PE warm-up on a never-written tile (values irrelevant, results
            # unread): no data dependency, so the PE is busy from ucode-load
            # time and the HAM clock-gate is at 2.4 GHz for the real matmuls.
            junk = cpool.tile([128, 2, H], BF16)
            nc.gpsimd.memset(junk[:], 0)

            def emit_load(i):
                a = apool.tile([128, 4, S], BF16, tag="a", name=f"a_{i}")
                nc.sync.dma_start(a[:], in_d[i])
                return a

            def emit_folds(i, a):
                """rowE = top + bottom (DVE, 2x add), rowO = top - bottom (Pool)."""
                rowE = rpool.tile([128, 2, S], BF16, tag="row", name=f"re_{i}")
                rowO = rpool.tile([128, 2, S], BF16, tag="row", name=f"ro_{i}")
                nc.vector.tensor_add(rowE[:], a[:, 0:2, :], a[:, 2:4, :])
                nc.gpsimd.tensor_sub(rowO[:], a[:, 0:2, :], a[:, 2:4, :])
                return rowE, rowO

            wu = ps2pool.tile([128, 2, 2, H], mybir.dt.float32, tag="ps2", name="warmup")
            for k in range(9):
                nc.tensor.matmul(
                    wu[:, k % 2, 0, :],
                    junk[:, 0, 0:128],
                    junk[:, k % 2, :],
                    start=True,
                    stop=True,
                )
            # ---- fast-path startup: image 0 loads as two fold-pair halves so
            # its folds (and first matmuls) start as early as possible; the
            # basis rides the idle Scalar DMA queue in parallel.
            a0A = a0pool.tile([128, 2, S], BF16, name="a0A")
            a0B = a0pool.tile([128, 2, S], BF16, name="a0B")
            nc.sync.dma_start(a0A[:], in_d[0, :, 0:3:2, :])  # groups 0, 2
            nc.scalar.dma_start(et_sb[:], et_d)
            nc.scalar.dma_start(ot_sb[:], ot_d)
            nc.sync.dma_start(a0B[:], in_d[0, :, 1:4:2, :])  # groups 1, 3
            a1 = emit_load(1)
            # image-0 folds, split per row-tile so t=0 follows the first DMA
            rowE0 = rpool.tile([128, 2, S], BF16, tag="row", name="re_0")
            rowO0 = rpool.tile([128, 2, S], BF16, tag="row", name="ro_0")
            nc.vector.tensor_add(rowE0[:, 0, :], a0A[:, 0, :], a0A[:, 1, :])
            nc.gpsimd.tensor_sub(rowO0[:, 0, :], a0A[:, 0, :], a0A[:, 1, :])
            nc.vector.tensor_add(rowE0[:, 1, :], a0B[:, 0, :], a0B[:, 1, :])
            nc.gpsimd.tensor_sub(rowO0[:, 1, :], a0B[:, 0, :], a0B[:, 1, :])
            pend = {1: a1}
            cur_folds = (rowE0, rowO0)
            prev_dt = None

            for i in range(BPC + 1):
                nxt_dt = None
                if i < BPC:
                    rowX = {"e": cur_folds[0], "o": cur_folds[1]}
                    # ---- stage 1: psL/psR per row-parity, [128, ns(2), 256]
                    ps = {}
                    for r in ("e", "o"):
                        src, b = rowX[r], bas[r]
                        pl = ps1pool.tile(
                            [128, 2, H], mybir.dt.float32, tag="ps1", name=f"pl_{i}_{r}"
                        )
                        pr = ps1pool.tile(
                            [128, 2, H], mybir.dt.float32, tag="ps1", name=f"pr_{i}_{r}"
                        )
                        for half, pst in ((0, pl), (1, pr)):
                            for ns in range(2):
                                for t in range(2):
                                    o = half * H + ns * 128
                                    nc.tensor.matmul(
                                        pst[:, ns, :],
                                        src[:, t, o : o + 128],
                                        b[:, t, :],
                                        start=(t == 0),
                                        stop=(t == 1),
                                    )
                        ps[r] = (pl, pr)
                    # prefetch + next image's row folds
                    if i + 2 < BPC:
                        pend[i + 2] = emit_load(i + 2)
                    if i + 1 < BPC:
                        cur_folds = emit_folds(i + 1, pend.pop(i + 1))
                    # ---- fused col-fold copies: dt_re = psL+psR, dt_ro = psL-psR
                    # (dual-PSUM tensor_tensor is illegal, so stage psL to SBUF
                    # f32 on ACT, then DVE combines SBUF + PSUM.)
                    nxt_dt = {}
                    for r in ("e", "o"):
                        pl, pr = ps[r]
                        sl = slpool.tile(
                            [128, 2, H], mybir.dt.float32, tag="sl", name=f"sl_{i}_{r}"
                        )
                        nc.scalar.copy(sl[:], pl[:])
                        da = dtpool.tile([128, 2, H], BF16, tag="dt", name=f"da_{i}_{r}")
                        ds = dtpool.tile([128, 2, H], BF16, tag="dt", name=f"ds_{i}_{r}")
                        nc.vector.tensor_add(da[:], sl[:], pr[:])
                        nc.vector.tensor_sub(ds[:], sl[:], pr[:])
                        nxt_dt[r] = (da, ds)

                if i >= 1:
                    # ---- stage 2 for image i-1: Y_rc[p', q']
                    j = i - 1
                    st = stpool.tile([128, 2, 2, 2, H], BF16, tag="st", name=f"st_{j}")
                    for pi, r in enumerate(("e", "o")):
                        da, ds = prev_dt[r]
                        p2 = ps2pool.tile(
                            [128, 2, 2, H], mybir.dt.float32, tag="ps2", name=f"p2_{j}_{r}"
                        )
                        for qi, (dtq, c) in enumerate(((da, "e"), (ds, "o"))):
                            for psl in range(2):
                                for t2 in range(2):
                                    nc.tensor.matmul(
                                        p2[:, qi, psl, :],
                                        dtq[:, t2, psl * 128 : (psl + 1) * 128],
                                        bas[c][:, t2, :],
                                        start=(t2 == 0),
                                        stop=(t2 == 1),
                                    )
                            if j == BPC - 1:
                                # tail drain: copy each quarter right after its
                                # matmuls, split across ACT/DVE, DMA per half
                                cp = nc.scalar.copy if pi == 0 else nc.vector.tensor_copy
                                cp(st[:, pi, qi], p2[:, qi])
                        if j == BPC - 1:
                            eng = nc.scalar if pi == 0 else nc.sync
                            eng.dma_start(out_d[j, :, pi], st[:, pi])
                        else:
                            nc.scalar.copy(st[:, pi], p2[:])
                    if j < BPC - 1:
                        nc.sync.dma_start(out_d[j], st[:])
                prev_dt = nxt_dt
    nc.compile()
    return nc


_NC_CACHE = None


def _get_nc():
    global _NC_CACHE
    if _NC_CACHE is None:
        _NC_CACHE = _build()
    return _NC_CACHE


def run_sharded(img: np.ndarray, **spmd_kwargs):
    """img [128, 1, 512, 512] f32 -> (out [128, 1, 512, 512] f32, results)."""
    img = np.asarray(img, dtype=np.float32).reshape(B_FULL, S, S)
    # host staging: permute rows+cols into fold order, tile rows into 4 groups
    x = img[:, PERM, :][:, :, PERM]
    xt = np.ascontiguousarray(
        x.reshape(B_FULL, 4, 128, S).transpose(0, 2, 1, 3)
    ).astype(ml_dtypes.bfloat16)  # [B, 128, 4, 512]
    et, ot = _basis_np()
    nc = _get_nc()
    in_maps = [
        {"inp": xt[k * BPC : (k + 1) * BPC], "et": et, "ot": ot}
        for k in range(N_CORES)
    ]
    res = run_bass_kernel_spmd(nc, in_maps, core_ids=list(range(N_CORES)), **spmd_kwargs)
    O = np.empty((B_FULL, 128, 2, 2, 2, H), dtype=np.float32)
    for k in range(N_CORES):
        O[k * BPC : (k + 1) * BPC] = np.asarray(res.results[k]["out"], dtype=np.float32)
    # O[b, u, r, c, ps, q'] = Y_rc[ps*128+u, q'] -> out[b, 2*(ps*128+u)+r, 2*q'+c]
    out = O.transpose(0, 4, 1, 2, 5, 3).reshape(B_FULL, S, S)
    return np.ascontiguousarray(out).reshape(B_FULL, 1, S, S), res


def kernel(img: np.ndarray) -> np.ndarray:
    out, _ = run_sharded(img)
    return out
